# revision 4
# baseline (speedup 1.0000x reference)
"""Trainium2 Bass kernel for a soft-target CrossEntropyLoss (log_softmax over
C=8 channels of [4, 8, 1024, 1024] logits, weighted masked mean).

Math: the reference reduces to a single scalar
    loss = -(1/(B*C*N)) * sum_{b,c,n} t[b,c,n] * x[b,c,n] * w[c]
with x = log_softmax(outs, axis=C) = o - lse  (lse = logsumexp over C), so
    sum t*x*w = P1 - P2,
    P1 = sum_{b,c,n} w_c * t * o
    P2 = sum_{b,n} lse[b,n] * s[b,n],   s = sum_c w_c * t

Sharding: H (1024 rows) split across 8 cores. Per (core, batch) the slab
[C=8, 128 rows, 1024] is *contiguously* viewed as [128 partitions, 8192]
with partition p = c*16 + q (q = 8-row block index). The 8 channel values of
any pixel live at partitions {c*16 + q : c}, i.e. p % 16 == q, so the
channel reduction is a matmul with a [128, 16] 0/1 selection matrix:
    SE[q, j] = sum_c exp(o)[c*16+q, j]     (lhsT = Sel)
    S [q, j] = sum_c w_c * t[c*16+q, j]    (lhsT = SelW = Sel * w[p//16])
exp/ln run on the scalar engine (same ACT table set). The final scalar
reductions are accumulating matmuls into two [1, 512] PSUM banks:
    P1 += w128^T @ (O * T)        (w128[p] = w[p//16])
    P2 += ones16^T @ (ln(SE) * S)
Host sums the 8 per-core (P1, P2) pairs.
"""

import sys

if "/opt/trn_rl_repo" not in sys.path:
    sys.path.insert(0, "/opt/trn_rl_repo")

import numpy as np

_B, _C, _H, _W = 4, 8, 1024, 1024
_M = 8                      # cores
_HC = _H // _M              # 128 rows per core
_P = 128                    # SBUF partitions
_FB = _C * _HC * _W // _P   # free elems per batch slab = 8192
_G = _P // _C               # 16 pixel groups per slab
_DEN = float(_B * _C * _H * _W)  # B*C*N = 33554432

_FD = 4096                  # DMA tile free size (2 MiB per tile)
_FS = 1024                  # compute subtile free size
_NMM = 512                  # matmul moving free size (one PSUM bank)

_CACHE = {}


def _build_nc():
    import concourse.bass as bass  # noqa: F401  (registers engines)
    import concourse.tile as tile
    from concourse import bacc, mybir

    f32 = mybir.dt.float32
    AF = mybir.ActivationFunctionType

    nc = bacc.Bacc(
        "TRN2",
        target_bir_lowering=False,
        debug=False,
        enable_asserts=False,
        num_devices=_M,
    )

    o_sh = nc.dram_tensor("o_sh", [_B, _P, _FB], f32, kind="ExternalInput").ap()
    t_sh = nc.dram_tensor("t_sh", [_B, _P, _FB], f32, kind="ExternalInput").ap()
    sel_d = nc.dram_tensor("sel", [_P, _G], f32, kind="ExternalInput").ap()
    selw_d = nc.dram_tensor("selw", [_P, _G], f32, kind="ExternalInput").ap()
    w128_d = nc.dram_tensor("w128", [_P, 1], f32, kind="ExternalInput").ap()
    ones_d = nc.dram_tensor("ones16", [_G, 1], f32, kind="ExternalInput").ap()
    res_d = nc.dram_tensor("res", [1, 2], f32, kind="ExternalOutput").ap()

    n_sub = _B * (_FB // _FD) * (_FD // _FS)  # total subtiles
    with tile.TileContext(nc, trace_sim=False) as tc:
        with (
            tc.tile_pool(name="consts", bufs=1) as consts,
            tc.tile_pool(name="o", bufs=3) as o_pool,
            tc.tile_pool(name="t", bufs=3) as t_pool,
            tc.tile_pool(name="e", bufs=3) as e_pool,
            tc.tile_pool(name="m", bufs=3) as m_pool,
            tc.tile_pool(name="l", bufs=3) as l_pool,
            tc.tile_pool(name="q", bufs=3) as q_pool,
            tc.tile_pool(name="res", bufs=1) as res_pool,
            tc.tile_pool(name="se", bufs=2, space="PSUM") as se_pool,
            tc.tile_pool(name="s", bufs=1, space="PSUM") as s_pool,
            tc.tile_pool(name="pacc", bufs=1, space="PSUM") as pacc_pool,
        ):
            sel_t = consts.tile([_P, _G], f32)
            nc.sync.dma_start(sel_t[:], sel_d)
            selw_t = consts.tile([_P, _G], f32)
            nc.sync.dma_start(selw_t[:], selw_d)
            w128_t = consts.tile([_P, 1], f32)
            nc.sync.dma_start(w128_t[:], w128_d)
            ones_t = consts.tile([_G, 1], f32)
            nc.sync.dma_start(ones_t[:], ones_d)

            p1_acc = pacc_pool.tile([1, _NMM], f32, tag="p1")
            p2_acc = pacc_pool.tile([1, _NMM], f32, tag="p2")

            sub = 0
            for b in range(_B):
                for d in range(_FB // _FD):
                    o_t = o_pool.tile([_P, _FD], f32)
                    nc.sync.dma_start(o_t[:], o_sh[b][:, d * _FD : (d + 1) * _FD])
                    t_t = t_pool.tile([_P, _FD], f32)
                    nc.sync.dma_start(t_t[:], t_sh[b][:, d * _FD : (d + 1) * _FD])
                    for k in range(_FD // _FS):
                        osub = o_t[:, k * _FS : (k + 1) * _FS]
                        tsub = t_t[:, k * _FS : (k + 1) * _FS]

                        e_t = e_pool.tile([_P, _FS], f32)
                        nc.scalar.activation(e_t[:], osub, AF.Exp)

                        m_t = m_pool.tile([_P, _FS], f32)
                        nc.vector.tensor_mul(m_t[:], osub, tsub)

                        se_t = se_pool.tile([_G, _FS], f32)
                        s_t = s_pool.tile([_G, _FS], f32)
                        for mm in range(_FS // _NMM):
                            ms = slice(mm * _NMM, (mm + 1) * _NMM)
                            nc.tensor.matmul(
                                se_t[:, ms], sel_t[:], e_t[:, ms],
                                start=True, stop=True,
                            )
                            nc.tensor.matmul(
                                s_t[:, ms], selw_t[:], tsub[:, ms],
                                start=True, stop=True,
                            )
                            nc.tensor.matmul(
                                p1_acc[:], w128_t[:], m_t[:, ms],
                                start=(sub == 0 and mm == 0),
                                stop=(sub == n_sub - 1 and mm == _FS // _NMM - 1),
                            )

                        l_t = l_pool.tile([_G, _FS], f32)
                        nc.scalar.activation(l_t[:], se_t[:], AF.Ln)

                        q_t = q_pool.tile([_G, _FS], f32)
                        nc.vector.tensor_mul(q_t[:], l_t[:], s_t[:])

                        for mm in range(_FS // _NMM):
                            ms = slice(mm * _NMM, (mm + 1) * _NMM)
                            nc.tensor.matmul(
                                p2_acc[:], ones_t[:], q_t[:, ms],
                                start=(sub == 0 and mm == 0),
                                stop=(sub == n_sub - 1 and mm == _FS // _NMM - 1),
                            )
                        sub += 1

            res_t = res_pool.tile([1, 2], f32)
            nc.vector.tensor_reduce(
                out=res_t[:, 0:1], in_=p1_acc[:],
                op=mybir.AluOpType.add, axis=mybir.AxisListType.X,
            )
            nc.vector.tensor_reduce(
                out=res_t[:, 1:2], in_=p2_acc[:],
                op=mybir.AluOpType.add, axis=mybir.AxisListType.X,
            )
            nc.sync.dma_start(res_d, res_t[:])

    nc.compile()
    return nc


def _get_nc():
    if "nc" not in _CACHE:
        _CACHE["nc"] = _build_nc()
    return _CACHE["nc"]


def _make_in_maps(outs, targets, class_weight):
    outs = np.asarray(outs, dtype=np.float32)
    targets = np.asarray(targets, dtype=np.float32)
    w = np.asarray(class_weight, dtype=np.float32)

    p = np.arange(_P)
    sel = (p[:, None] % _G == np.arange(_G)[None, :]).astype(np.float32)
    selw = sel * w[p // _G][:, None]
    w128 = np.ascontiguousarray(w[p // _G][:, None])
    ones16 = np.ones((_G, 1), dtype=np.float32)

    in_maps = []
    for k in range(_M):
        o_k = np.ascontiguousarray(
            outs[:, :, k * _HC : (k + 1) * _HC, :]
        ).reshape(_B, _P, _FB)
        t_k = np.ascontiguousarray(
            targets[:, :, k * _HC : (k + 1) * _HC, :]
        ).reshape(_B, _P, _FB)
        in_maps.append(
            {
                "o_sh": o_k,
                "t_sh": t_k,
                "sel": sel,
                "selw": selw,
                "w128": w128,
                "ones16": ones16,
            }
        )
    return in_maps


def _run(in_maps, trace=False):
    from concourse.bass_utils import run_bass_kernel_spmd

    nc = _get_nc()
    return run_bass_kernel_spmd(nc, in_maps, list(range(_M)), trace=trace)


def _combine(results, logits_input):
    p1 = 0.0
    p2 = 0.0
    for r in results:
        p1 += float(r["res"][0, 0])
        p2 += float(r["res"][0, 1])
    if logits_input:
        return np.float32((p2 - p1) / _DEN)
    return np.float32(-p1 / _DEN)


def kernel(outs, targets, class_weight, logits_input):
    logits = bool(np.asarray(logits_input).item())
    in_maps = _make_in_maps(outs, targets, class_weight)
    res = _run(in_maps, trace=False)
    return _combine(res.results, logits)
